# revision 22
# baseline (speedup 1.0000x reference)
"""Sigmoid-attention MHA kernel for 8 Trainium2 NeuronCores (v2).

Problem: x[4,2048,512], W_q/W_k/W_v/W_o[512,512] (already scaled).
  Q = x@Wq.T, K = x@Wk.T, V = x@Wv.T split into 8 heads of depth 64
  attn = sigmoid(QK^T/sqrt(64) - log(2048));  out = (attn@V merged)@Wo.T

Sharding: core c handles batch b=c//2, head-group g=c%2 (4 heads each).
Each core computes a partial output projection over its 256 head-features;
host sums the two partials per batch.

v2 engine plan (per core):
  PE      scores (fp32r, 2 heads/tile), attn@V as attn-stationary x
          V-moving bf16 (64-row moving passes), O-block transposes,
          Q/K/V/Wo projections.
  ScalarE sigmoid on ~(16-NDVE)/16 of score tiles (PSUM->SBUF bf16).
  Pool    Schraudolph affine->int32 (e^-z bits) for the DVE tiles,
          psO->bf16 staging copies, Wo wave copies.
  DVE     +1.0 and fused reciprocal (custom op) to finish sigmoid on
          NDVE/16 tiles, psT->ot copies, projection copies.
  attn@V is emitted DELAY tiles behind scores so the long DVE sigmoid
  latency never stalls the PE.
"""

import os
import numpy as np

DEBUG = bool(int(os.environ.get("KERNEL_DEBUG", "0")))
LOOP = int(os.environ.get("KERNEL_LOOP", "0"))  # >0: wrap body in For_i (timing)
NDVE = int(os.environ.get("KERNEL_NDVE", "4"))  # kc%16 < NDVE -> DVE sigmoid
DELAY = int(os.environ.get("KERNEL_DELAY", "5"))  # attnV emission delay (tiles)
ATTNV = os.environ.get("KERNEL_ATTNV", "movv")  # movv | mova
I2ENG = os.environ.get("KERNEL_I2ENG", "pool")  # dve | pool
ABUFS = int(os.environ.get("KERNEL_ABUFS", "6"))

B, S, D = 4, 2048, 512
NH, DEPTH = 8, 64
G = 2          # head groups (one per core pair)
GF = 256       # features per group
NEG_LOG_S = float(np.float32(-np.log(np.float32(S))))
INV_SQRT_DK = 0.125

# Schraudolph constants: bits(e^-z) ~= round(s*SC0 + SC1), z = s/8 - log(S)
A_LN2 = float(2**23 / np.log(2))
CMAGIC = 486411.0
SC0 = float(np.float32(-A_LN2 / 8.0))
SC1 = float(np.float32(A_LN2 * np.log(float(S)) + 127 * 2**23 - CMAGIC))

_CACHE = {}


def _build_nc():
    import concourse.bacc as bacc
    import concourse.tile as tile
    from concourse import mybir
    from concourse.alu_op_type import AluOpType
    from concourse.dve_ops import RECIP_APPROX_FAST_CONSTS, RECIPROCAL_APPROX_FAST

    f32 = mybir.dt.float32
    f32r = mybir.dt.float32r
    bf16 = mybir.dt.bfloat16
    i32 = mybir.dt.int32
    RC = RECIP_APPROX_FAST_CONSTS
    nc = bacc.Bacc("TRN2", target_bir_lowering=False, debug=False, num_devices=8)

    xt_d = nc.dram_tensor("xt", [128, 8192], f32r, kind="ExternalInput").ap()
    wq_d = nc.dram_tensor("wq", [128, 1024], f32r, kind="ExternalInput").ap()
    wk_d = nc.dram_tensor("wk", [128, 1024], f32r, kind="ExternalInput").ap()
    wv_d = nc.dram_tensor("wv", [128, 1024], f32r, kind="ExternalInput").ap()
    wo_d = nc.dram_tensor("wo", [128, 1024], f32, kind="ExternalInput").ap()
    id_d = nc.dram_tensor("ident", [128, 128], f32, kind="ExternalInput").ap()
    out_d = nc.dram_tensor("out", [S, D], f32, kind="ExternalOutput").ap()
    dbg = {}
    if DEBUG:
        for nm in ("qt", "kt"):
            dbg[nm] = [nc.dram_tensor(f"dbg_{nm}{m}", [128, 2048], f32r,
                                      kind="ExternalOutput").ap() for m in range(2)]
        dbg["ot"] = [nc.dram_tensor(f"dbg_ot{m}", [128, 2048], f32,
                                    kind="ExternalOutput").ap() for m in range(2)]
        dbg["v"] = [nc.dram_tensor(f"dbg_v{t}", [128, 256], f32,
                                   kind="ExternalOutput").ap() for t in range(16)]

    with tile.TileContext(nc) as tc:
        with (
            tc.tile_pool(name="persist", bufs=1) as persist,
            tc.tile_pool(name="attn", bufs=ABUFS) as apool,
            tc.tile_pool(name="epool", bufs=3) as epool,
            tc.tile_pool(name="stage", bufs=3) as stage,
            tc.tile_pool(name="spsum", bufs=3, space="PSUM") as spsum,
            tc.tile_pool(name="opsum", bufs=1, space="PSUM") as opsum,
            tc.tile_pool(name="mpsum", bufs=1, space="PSUM") as mpsum,
        ):
            import contextlib
            if LOOP > 0:
                loop_cm = tc.For_i(0, LOOP, 1)
            else:
                loop_cm = contextlib.nullcontext()
            Sig = mybir.ActivationFunctionType.Sigmoid

            def mm(out, lhsT, rhs, start, stop):
                # f32r: single-pass fp32 matmul (4x faster than fp32 on PE)
                nc.tensor.matmul(out, lhsT=lhsT.bitcast(f32r),
                                 rhs=rhs.bitcast(f32r), start=start, stop=stop)

            def mmb(out, lhsT, rhs, start, stop):
                nc.tensor.matmul(out, lhsT=lhsT, rhs=rhs, start=start,
                                 stop=stop, skip_group_check=True)

            with loop_cm:
                bias_t = persist.tile([128, 1], f32, tag="bias", name="bias_t")
                nc.vector.memset(bias_t[:], NEG_LOG_S)
                warm_t = persist.tile([128, 1], f32, tag="warm", name="warm_t")
                nc.scalar.activation(warm_t[:], bias_t[:], Sig, bias=bias_t[:])

                wq_sb = persist.tile([128, 1024], f32r, tag="wq", name="wq_sb")
                wk_sb = persist.tile([128, 1024], f32r, tag="wk", name="wk_sb")
                wv_sb = persist.tile([128, 1024], f32r, tag="wv", name="wv_sb")
                wo_f = persist.tile([128, 1024], f32, tag="wof", name="wo_f")
                wo_sb = persist.tile([128, 1024], bf16, tag="wo", name="wo_sb")
                id_f = persist.tile([128, 128], f32, tag="idf", name="id_f")
                idb = persist.tile([128, 128], bf16, tag="idb", name="idb")
                xt = [persist.tile([128, 2048], f32r, tag=f"xt{c}", name=f"xt{c}")
                      for c in range(4)]
                nc.sync.dma_start(out=xt[0][:], in_=xt_d[:, 0:2048])
                nc.scalar.dma_start(out=xt[1][:], in_=xt_d[:, 2048:4096])
                nc.gpsimd.dma_start(out=xt[2][:], in_=xt_d[:, 4096:6144])
                nc.sync.dma_start(out=xt[3][:], in_=xt_d[:, 6144:8192])
                nc.scalar.dma_start(out=wq_sb[:], in_=wq_d[:])
                nc.gpsimd.dma_start(out=wk_sb[:], in_=wk_d[:])
                nc.sync.dma_start(out=wv_sb[:], in_=wv_d[:])
                nc.scalar.dma_start(out=wo_f[:], in_=wo_d[:])
                nc.gpsimd.dma_start(out=id_f[:], in_=id_d[:])
                nc.gpsimd.tensor_copy(wo_sb[:], wo_f[:])
                nc.gpsimd.tensor_copy(idb[:], id_f[:])

                qt = [persist.tile([128, 2048], f32r, tag=f"qt{m}", name=f"qt{m}")
                      for m in range(2)]
                kt = [persist.tile([128, 2048], f32r, tag=f"kt{m}", name=f"kt{m}")
                      for m in range(2)]
                v = [persist.tile([128, 512], bf16, tag=f"v{t}", name=f"v{t}")
                     for t in range(8)]
                ot = [persist.tile([128, 2048], bf16, tag=f"ot{m}", name=f"ot{m}")
                      for m in range(2)]
                otmp = [persist.tile([64, 512], bf16, tag=f"otmp{m}",
                                     name=f"otmp{m}") for m in range(8)]

                # ---- Q/K projection chain (emitted just-in-time) ----
                pi = [0]

                def proj_chain(which, mc, qc):
                    w_sb = (wq_sb, wk_sb)[which]
                    dst = (qt, kt)[which][mc]
                    ps = mpsum.tile([128, 512], f32, tag="m", name="psP")
                    for kc in range(4):
                        mm(ps[:, 0:512],
                           w_sb[:, 256 * kc + 128 * mc:256 * kc + 128 * mc + 128],
                           xt[kc][:, 512 * qc:512 * (qc + 1)],
                           start=(kc == 0), stop=(kc == 3))
                    if pi[0] % 2 == 0:
                        nc.scalar.copy(dst[:, 512 * qc:512 * (qc + 1)],
                                       ps[:, 0:512])
                    else:
                        nc.vector.tensor_copy(
                            dst[:, 512 * qc:512 * (qc + 1)], ps[:, 0:512])
                    pi[0] += 1

                # ---- attention ----
                def emit_v_pair(t2):
                    # V proj for token chunks 2*t2, 2*t2+1 into one psum slot
                    pv = spsum.tile([128, 1024], f32, tag="s", name="ps")
                    for half in range(2):
                        tck = 2 * t2 + half
                        cs = slice(256 * half, 256 * (half + 1))
                        for vkc in range(4):
                            mm(pv[:, cs],
                               xt[vkc][:, 128 * tck:128 * (tck + 1)],
                               wv_sb[:, 256 * vkc:256 * (vkc + 1)],
                               start=(vkc == 0 and half == 0),
                               stop=(vkc == 3 and half == 1))
                    nc.vector.tensor_copy(v[t2][:], pv[:, 0:512])

                def emit_scores(p, qc, kc):
                    ks = slice(128 * kc, 128 * (kc + 1))
                    qs = slice(512 * qc, 512 * (qc + 1))
                    s = spsum.tile([128, 1024], f32, tag="s", name="ps")
                    mm(s[:, 0:512], kt[p][0:64, ks], qt[p][0:64, qs],
                       start=True, stop=True)
                    mm(s[:, 512:1024], kt[p][64:128, ks],
                       qt[p][64:128, qs], start=True, stop=True)
                    return s

                def emit_sigmoid_act(s):
                    a = apool.tile([128, 1024], bf16, tag="a", name="attn")
                    nc.scalar.activation(a[:], s[:], Sig,
                                         bias=bias_t[:], scale=INV_SQRT_DK)
                    return a

                def emit_sigmoid_dve12(s):
                    # e^{-z} bits via Schraudolph affine -> int32, then +1.0
                    ei = epool.tile([128, 1024], i32, tag="ei", name="ei")
                    nc.vector.tensor_scalar(ei[:], s[:], SC0, SC1,
                                            AluOpType.mult, AluOpType.add)
                    y = epool.tile([128, 1024], f32, tag="y", name="y")
                    i2 = nc.vector if I2ENG == "dve" else nc.gpsimd
                    i2.tensor_scalar_add(y[:], ei[:].bitcast(f32), 1.0)
                    return y

                def emit_sigmoid_dve3(y):
                    # sigma = 1/(1+e^{-z})
                    a = apool.tile([128, 1024], bf16, tag="a", name="attn")
                    nc.vector._custom_dve(RECIPROCAL_APPROX_FAST, out=a[:],
                                          in0=y[:], s0=RC["s0"],
                                          s1=RC["s1"], imm2=RC["imm2"])
                    return a

                psO = [None]

                def attn_v_movv(p, qc, kc, a):
                    # returns obf staging tile at group end (kc==15)
                    if kc == 0:
                        psO[0] = opsum.tile([128, 512], f32, tag="o", name="psO")
                    for j in range(4):
                        for h in range(2):
                            mmb(psO[0][:, 128 * j + 64 * h:128 * j + 64 * h + 64],
                                a[:, 512 * h + 128 * j:512 * h + 128 * j + 128],
                                v[kc // 2][:, 256 * (kc % 2) + 128 * p + 64 * h:
                                            256 * (kc % 2) + 128 * p + 64 * h + 64],
                                start=(kc == 0 and j == 0 and h == 0),
                                stop=(kc == 15 and j == 3 and h == 1))
                    if kc == 15:
                        obf = stage.tile([128, 512], bf16, tag="obf", name="obf")
                        nc.vector.tensor_copy(obf[:], psO[0][:])
                        return obf
                    return None

                def emit_transposes(obf):
                    psT = mpsum.tile([128, 512], f32, tag="m",
                                     name="psT")[:].bitcast(bf16)
                    for j in range(4):
                        nc.tensor.matmul(psT[:, 128 * j:128 * (j + 1)],
                                         lhsT=obf[:, 128 * j:128 * (j + 1)],
                                         rhs=idb[:], is_transpose=True,
                                         start=(j == 0), stop=(j == 3),
                                         skip_group_check=True)
                    return psT

                def emit_ot_copy(p, qc, psT):
                    qs = slice(512 * qc, 512 * (qc + 1))
                    nc.vector.tensor_copy(ot[p][:, qs], psT[:, 0:512])

                def attn_v_mova(p, qc, kc, a):
                    if kc == 0:
                        psO[0] = opsum.tile([64, 1024], f32, tag="o2", name="psO2")
                    vb = v[kc // 2][:, 256 * (kc % 2):256 * (kc % 2 + 1)]
                    mmb(psO[0][0:64, 0:512], vb[:, 128 * p:128 * p + 64],
                        a[:, 0:512], start=(kc == 0), stop=(kc == 15))
                    mmb(psO[0][0:64, 512:1024],
                        vb[:, 128 * p + 64:128 * p + 128],
                        a[:, 512:1024], start=(kc == 0), stop=(kc == 15))
                    if kc == 15:
                        qs = slice(512 * qc, 512 * (qc + 1))
                        nc.vector.tensor_copy(ot[p][0:64, qs], psO[0][0:64, 0:512])
                        tmp = otmp[4 * p + qc]
                        nc.vector.tensor_copy(tmp[0:64, :], psO[0][0:64, 512:1024])
                        nc.sync.dma_start(out=ot[p][64:128, qs], in_=tmp[0:64, :])

                attn_v = attn_v_movv if ATTNV == "movv" else attn_v_mova

                wave_n = [0]

                def emit_wave(qc, w):
                    # tokens 512qc+128w : out rows <- ot[:, block] @ wo
                    tk = slice(512 * qc + 128 * w, 512 * qc + 128 * (w + 1))
                    psW = mpsum.tile([128, 512], f32, tag="m", name="psW")
                    for c in range(2):
                        mmb(psW[:], ot[c][:, tk], wo_sb[:, 512 * c:512 * (c + 1)],
                            start=(c == 0), stop=(c == 1))
                    st = stage.tile([128, 512], f32, tag="st", name="st")
                    nc.vector.tensor_copy(st[:], psW[:])
                    q = (nc.sync, nc.gpsimd)[wave_n[0] % 2]
                    wave_n[0] += 1
                    q.dma_start(out=out_d[tk, :], in_=st[:])

                # ---- main software-pipelined loop ----
                flat = [(qc, p, kc) for qc in range(4) for p in range(2)
                        for kc in range(16)]
                T = len(flat)
                dve_set = {kc for kc in range(16)
                           if (kc + 1) * NDVE // 16 > kc * NDVE // 16}

                # prologue: only the chains scores(0) needs; rest are
                # emitted just-before-deadline inside the loop.
                proj_chain(0, 0, 0)   # qt0 qc0
                proj_chain(1, 0, 0)   # kt0 qc0
                chain_deadlines = (
                    [(1, 0, qcK, 4 * qcK) for qcK in range(1, 4)] +
                    [(0, 1, 0, 16)] +
                    [(1, 1, qcK, 16 + 4 * qcK) for qcK in range(4)] +
                    [(0, 0, 1, 32), (0, 1, 1, 48), (0, 0, 2, 64),
                     (0, 1, 2, 80), (0, 0, 3, 96), (0, 1, 3, 112)])
                sched = {}
                for which, mc, qcK, dl in chain_deadlines:
                    sched.setdefault(max(0, dl - 4), []).append(
                        (lambda w=which, m=mc, q=qcK: proj_chain(w, m, q)))

                attn_q = {}
                y_pend = {}
                obf_pend = {}
                ot_pend = {}
                wave_q = []
                for i in range(T + DELAY + 8):
                    # 1. attnV for the delayed tile (PE) + group-end obf (DVE)
                    j = i - DELAY
                    if DELAY <= i < T + DELAY:
                        qc, p, kc = flat[j]
                        obf = attn_v(p, qc, kc, attn_q.pop(j))
                        if obf is not None:
                            obf_pend[i + 1] = (p, qc, obf)
                            if p == 1:
                                wave_q.extend((i + 3 + 2 * w, qc, w)
                                              for w in range(4))
                    # 2. finish DVE sigmoid of previous tile (DVE ready now)
                    if i - 1 in y_pend:
                        attn_q[i - 1] = emit_sigmoid_dve3(y_pend.pop(i - 1))
                    # 3. new scores + sigmoid start
                    if i < T:
                        qc, p, kc = flat[i]
                        s = emit_scores(p, qc, kc)
                        if kc in dve_set:
                            y_pend[i] = emit_sigmoid_dve12(s)
                        else:
                            attn_q[i] = emit_sigmoid_act(s)
                    # 4. group-end pipeline: transposes, then ot copy
                    if i in obf_pend:
                        p2, q2, o2 = obf_pend.pop(i)
                        ot_pend[i + 1] = (p2, q2, emit_transposes(o2))
                    if i in ot_pend:
                        emit_ot_copy(*ot_pend.pop(i))
                    # 5. scheduled proj chains
                    for fn in sched.pop(i, []):
                        fn()
                    # 6. V projection pairs (first 16 tiles, every other)
                    if i < 16 and i % 2 == 0:
                        emit_v_pair(i // 2)
                    # 7. output waves
                    if wave_q and i >= wave_q[0][0]:
                        emit_wave(*wave_q.pop(0)[1:])
                while wave_q:
                    emit_wave(*wave_q.pop(0)[1:])

                if DEBUG:
                    for m in range(2):
                        nc.sync.dma_start(out=dbg["qt"][m], in_=qt[m][:])
                        nc.sync.dma_start(out=dbg["kt"][m], in_=kt[m][:])
                    for m in range(2):
                        otf = persist.tile([128, 2048], f32, tag=f"otf{m}",
                                           name=f"otf{m}")
                        nc.vector.tensor_copy(otf[:], ot[m][:])
                        nc.sync.dma_start(out=dbg["ot"][m], in_=otf[:])
                    for t in range(8):
                        vf = persist.tile([128, 512], f32, tag=f"vf{t}",
                                          name=f"vf{t}")
                        nc.vector.tensor_copy(vf[:], v[t][:])
                        nc.sync.dma_start(out=dbg["v"][2 * t], in_=vf[:, 0:256])
                        nc.sync.dma_start(out=dbg["v"][2 * t + 1],
                                          in_=vf[:, 256:512])

    nc.compile()
    return nc


def get_nc():
    if "nc" not in _CACHE:
        _CACHE["nc"] = _build_nc()
    return _CACHE["nc"]


def make_in_maps(x, W_q, W_k, W_v, W_o):
    x = np.ascontiguousarray(np.asarray(x, dtype=np.float32))
    ws = [np.asarray(w, dtype=np.float32) for w in (W_q, W_k, W_v, W_o)]
    W_q, W_k, W_v, W_o = ws

    def chunked(a, nchunks):
        # [128*nchunks, m] -> [128, nchunks*m] with chunk-major columns
        m = a.shape[1]
        return np.ascontiguousarray(
            a.reshape(nchunks, 128, m).transpose(1, 0, 2).reshape(128, nchunks * m))

    ident = np.ascontiguousarray(np.eye(128, dtype=np.float32))
    in_maps = []
    for c in range(8):
        b, g = divmod(c, 2)
        gf = slice(GF * g, GF * (g + 1))
        in_maps.append({
            "xt": chunked(np.ascontiguousarray(x[b].T), 4),
            "wq": chunked(np.ascontiguousarray(W_q[gf, :].T), 4),
            "wk": chunked(np.ascontiguousarray(W_k[gf, :].T), 4),
            "wv": chunked(np.ascontiguousarray(W_v[gf, :].T), 4),
            "wo": chunked(np.ascontiguousarray(W_o[:, gf].T), 2),
            "ident": ident,
        })
    return in_maps


def kernel(x, W_q, W_k, W_v, W_o):
    from concourse.bass_utils import run_bass_kernel_spmd

    nc = get_nc()
    in_maps = make_in_maps(x, W_q, W_k, W_v, W_o)
    res = run_bass_kernel_spmd(nc, in_maps, list(range(8)))
    parts = [res.results[c]["out"] for c in range(8)]
    out = np.stack([parts[2 * b] + parts[2 * b + 1] for b in range(B)])
    return np.ascontiguousarray(out.astype(np.float32))


# revision 25
# speedup vs baseline: 12.5934x; 12.5934x over previous
"""Sigmoid-attention MHA kernel for 8 Trainium2 NeuronCores (v2).

Problem: x[4,2048,512], W_q/W_k/W_v/W_o[512,512] (already scaled).
  Q = x@Wq.T, K = x@Wk.T, V = x@Wv.T split into 8 heads of depth 64
  attn = sigmoid(QK^T/sqrt(64) - log(2048));  out = (attn@V merged)@Wo.T

Sharding: core c handles batch b=c//2, head-group g=c%2 (4 heads each).
Each core computes a partial output projection over its 256 head-features;
host sums the two partials per batch.

v2 engine plan (per core):
  PE      scores (fp32r, 2 heads/tile), attn@V as attn-stationary x
          V-moving bf16 (64-row moving passes), O-block transposes,
          Q/K/V/Wo projections.
  ScalarE sigmoid on ~(16-NDVE)/16 of score tiles (PSUM->SBUF bf16).
  Pool    Schraudolph affine->int32 (e^-z bits) for the DVE tiles,
          psO->bf16 staging copies, Wo wave copies.
  DVE     +1.0 and fused reciprocal (custom op) to finish sigmoid on
          NDVE/16 tiles, psT->ot copies, projection copies.
  attn@V is emitted DELAY tiles behind scores so the long DVE sigmoid
  latency never stalls the PE.
"""

import os
import numpy as np

DEBUG = bool(int(os.environ.get("KERNEL_DEBUG", "0")))
LOOP = int(os.environ.get("KERNEL_LOOP", "0"))  # >0: wrap body in For_i (timing)
NDVE = int(os.environ.get("KERNEL_NDVE", "0"))  # kc%16 < NDVE -> DVE sigmoid
DELAY = int(os.environ.get("KERNEL_DELAY", "5"))  # attnV emission delay (tiles)
ATTNV = os.environ.get("KERNEL_ATTNV", "movv")  # movv | mova
I2ENG = os.environ.get("KERNEL_I2ENG", "pool")  # dve | pool
ABUFS = int(os.environ.get("KERNEL_ABUFS", "6"))
GPDMA = bool(int(os.environ.get("KERNEL_GPDMA", "0")))

B, S, D = 4, 2048, 512
NH, DEPTH = 8, 64
G = 2          # head groups (one per core pair)
GF = 256       # features per group
NEG_LOG_S = float(np.float32(-np.log(np.float32(S))))
INV_SQRT_DK = 0.125

# Schraudolph constants: bits(e^-z) ~= round(s*SC0 + SC1), z = s/8 - log(S)
A_LN2 = float(2**23 / np.log(2))
CMAGIC = 486411.0
SC0 = float(np.float32(-A_LN2 / 8.0))
SC1 = float(np.float32(A_LN2 * np.log(float(S)) + 127 * 2**23 - CMAGIC))

_CACHE = {}


def _build_nc():
    import concourse.bacc as bacc
    import concourse.tile as tile
    from concourse import mybir
    from concourse.alu_op_type import AluOpType
    from concourse.dve_ops import RECIP_APPROX_FAST_CONSTS, RECIPROCAL_APPROX_FAST

    f32 = mybir.dt.float32
    f32r = mybir.dt.float32r
    bf16 = mybir.dt.bfloat16
    i32 = mybir.dt.int32
    RC = RECIP_APPROX_FAST_CONSTS
    nc = bacc.Bacc("TRN2", target_bir_lowering=False, debug=False, num_devices=8)

    xt_d = nc.dram_tensor("xt", [128, 8192], f32r, kind="ExternalInput").ap()
    wq_d = nc.dram_tensor("wq", [128, 1024], f32r, kind="ExternalInput").ap()
    wk_d = nc.dram_tensor("wk", [128, 1024], f32r, kind="ExternalInput").ap()
    wv_d = nc.dram_tensor("wv", [128, 1024], f32r, kind="ExternalInput").ap()
    wo_d = nc.dram_tensor("wo", [128, 1024], f32, kind="ExternalInput").ap()
    id_d = nc.dram_tensor("ident", [128, 128], f32, kind="ExternalInput").ap()
    out_d = nc.dram_tensor("out", [S, D], f32, kind="ExternalOutput").ap()
    dbg = {}
    if DEBUG:
        for nm in ("qt", "kt"):
            dbg[nm] = [nc.dram_tensor(f"dbg_{nm}{m}", [128, 2048], f32r,
                                      kind="ExternalOutput").ap() for m in range(2)]
        dbg["ot"] = [nc.dram_tensor(f"dbg_ot{m}", [128, 2048], f32,
                                    kind="ExternalOutput").ap() for m in range(2)]
        dbg["v"] = [nc.dram_tensor(f"dbg_v{t}", [128, 256], f32,
                                   kind="ExternalOutput").ap() for t in range(16)]

    with tile.TileContext(nc) as tc:
        with (
            tc.tile_pool(name="persist", bufs=1) as persist,
            tc.tile_pool(name="attn", bufs=ABUFS) as apool,
            tc.tile_pool(name="epool", bufs=3) as epool,
            tc.tile_pool(name="stage", bufs=3) as stage,
            tc.tile_pool(name="spsum", bufs=3 if ATTNV == "movv" else 2,
                         space="PSUM") as spsum,
            tc.tile_pool(name="opsum", bufs=1, space="PSUM") as opsum,
            tc.tile_pool(name="mpsum", bufs=1, space="PSUM") as mpsum,
        ):
            import contextlib
            if LOOP > 0:
                loop_cm = tc.For_i(0, LOOP, 1)
            else:
                loop_cm = contextlib.nullcontext()
            Sig = mybir.ActivationFunctionType.Sigmoid

            def mm(out, lhsT, rhs, start, stop):
                # f32r: single-pass fp32 matmul (4x faster than fp32 on PE)
                nc.tensor.matmul(out, lhsT=lhsT.bitcast(f32r),
                                 rhs=rhs.bitcast(f32r), start=start, stop=stop)

            def mmb(out, lhsT, rhs, start, stop):
                nc.tensor.matmul(out, lhsT=lhsT, rhs=rhs, start=start,
                                 stop=stop, skip_group_check=True)

            with loop_cm:
                bias_t = persist.tile([128, 1], f32, tag="bias", name="bias_t")
                nc.vector.memset(bias_t[:], NEG_LOG_S)
                warm_t = persist.tile([128, 1], f32, tag="warm", name="warm_t")
                nc.scalar.activation(warm_t[:], bias_t[:], Sig, bias=bias_t[:])

                wq_sb = persist.tile([128, 1024], f32r, tag="wq", name="wq_sb")
                wk_sb = persist.tile([128, 1024], f32r, tag="wk", name="wk_sb")
                wv_sb = persist.tile([128, 1024], f32r, tag="wv", name="wv_sb")
                wo_f = persist.tile([128, 1024], f32, tag="wof", name="wo_f")
                wo_sb = persist.tile([128, 1024], bf16, tag="wo", name="wo_sb")
                id_f = persist.tile([128, 128], f32, tag="idf", name="id_f")
                idb = persist.tile([128, 128], bf16, tag="idb", name="idb")
                xt = [persist.tile([128, 2048], f32r, tag=f"xt{c}", name=f"xt{c}")
                      for c in range(4)]
                q3 = nc.gpsimd if GPDMA else nc.sync
                nc.sync.dma_start(out=xt[0][:], in_=xt_d[:, 0:2048])
                nc.scalar.dma_start(out=xt[1][:], in_=xt_d[:, 2048:4096])
                q3.dma_start(out=xt[2][:], in_=xt_d[:, 4096:6144])
                nc.sync.dma_start(out=xt[3][:], in_=xt_d[:, 6144:8192])
                nc.scalar.dma_start(out=wq_sb[:], in_=wq_d[:])
                q3.dma_start(out=wk_sb[:], in_=wk_d[:])
                nc.sync.dma_start(out=wv_sb[:], in_=wv_d[:])
                nc.scalar.dma_start(out=wo_f[:], in_=wo_d[:])
                q3.dma_start(out=id_f[:], in_=id_d[:])
                nc.vector.tensor_copy(wo_sb[:], wo_f[:])
                nc.vector.tensor_copy(idb[:], id_f[:])

                qt = [persist.tile([128, 2048], f32r, tag=f"qt{m}", name=f"qt{m}")
                      for m in range(2)]
                kt = [persist.tile([128, 2048], f32r, tag=f"kt{m}", name=f"kt{m}")
                      for m in range(2)]
                v = [persist.tile([128, 512], bf16, tag=f"v{t}", name=f"v{t}")
                     for t in range(8)]
                ot = [persist.tile([128, 2048], bf16, tag=f"ot{m}", name=f"ot{m}")
                      for m in range(2)]
                otmp = [persist.tile([64, 512], bf16, tag=f"otmp{m}",
                                     name=f"otmp{m}") for m in range(8)]

                # ---- Q/K projection chain (emitted just-in-time) ----
                pi = [0]

                def proj_chain(which, mc, qc):
                    w_sb = (wq_sb, wk_sb)[which]
                    dst = (qt, kt)[which][mc]
                    ps = mpsum.tile([128, 512], f32, tag="m", name="psP")
                    for kc in range(4):
                        mm(ps[:, 0:512],
                           w_sb[:, 256 * kc + 128 * mc:256 * kc + 128 * mc + 128],
                           xt[kc][:, 512 * qc:512 * (qc + 1)],
                           start=(kc == 0), stop=(kc == 3))
                    nc.vector.tensor_copy(
                        dst[:, 512 * qc:512 * (qc + 1)], ps[:, 0:512])
                    pi[0] += 1

                # ---- attention ----
                def emit_v_pair(t2):
                    # V proj for token chunks 2*t2, 2*t2+1 into one psum slot
                    pv = spsum.tile([128, 1024], f32, tag="s", name="ps")
                    for half in range(2):
                        tck = 2 * t2 + half
                        cs = slice(256 * half, 256 * (half + 1))
                        for vkc in range(4):
                            mm(pv[:, cs],
                               xt[vkc][:, 128 * tck:128 * (tck + 1)],
                               wv_sb[:, 256 * vkc:256 * (vkc + 1)],
                               start=(vkc == 0 and half == 0),
                               stop=(vkc == 3 and half == 1))
                    nc.vector.tensor_copy(v[t2][:], pv[:, 0:512])

                def emit_scores(p, qc, kc):
                    ks = slice(128 * kc, 128 * (kc + 1))
                    qs = slice(512 * qc, 512 * (qc + 1))
                    s = spsum.tile([128, 1024], f32, tag="s", name="ps")
                    mm(s[:, 0:512], kt[p][0:64, ks], qt[p][0:64, qs],
                       start=True, stop=True)
                    mm(s[:, 512:1024], kt[p][64:128, ks],
                       qt[p][64:128, qs], start=True, stop=True)
                    return s

                def emit_sigmoid_act(s):
                    a = apool.tile([128, 1024], bf16, tag="a", name="attn")
                    nc.scalar.activation(a[:], s[:], Sig,
                                         bias=bias_t[:], scale=INV_SQRT_DK)
                    return a

                def emit_sigmoid_dve12(s):
                    # e^{-z} bits via Schraudolph affine -> int32, then +1.0
                    ei = epool.tile([128, 1024], i32, tag="ei", name="ei")
                    nc.vector.tensor_scalar(ei[:], s[:], SC0, SC1,
                                            AluOpType.mult, AluOpType.add)
                    y = epool.tile([128, 1024], f32, tag="y", name="y")
                    i2 = nc.vector if I2ENG == "dve" else nc.gpsimd
                    i2.tensor_scalar_add(y[:], ei[:].bitcast(f32), 1.0)
                    return y

                def emit_sigmoid_dve3(y):
                    # sigma = 1/(1+e^{-z})
                    a = apool.tile([128, 1024], bf16, tag="a", name="attn")
                    nc.vector._custom_dve(RECIPROCAL_APPROX_FAST, out=a[:],
                                          in0=y[:], s0=RC["s0"],
                                          s1=RC["s1"], imm2=RC["imm2"])
                    return a

                psO = [None]

                def attn_v_movv(p, qc, kc, a):
                    # returns obf staging tile at group end (kc==15)
                    if kc == 0:
                        psO[0] = opsum.tile([128, 512], f32, tag="o", name="psO")
                    for j in range(4):
                        for h in range(2):
                            mmb(psO[0][:, 128 * j + 64 * h:128 * j + 64 * h + 64],
                                a[:, 512 * h + 128 * j:512 * h + 128 * j + 128],
                                v[kc // 2][:, 256 * (kc % 2) + 128 * p + 64 * h:
                                            256 * (kc % 2) + 128 * p + 64 * h + 64],
                                start=(kc == 0 and j == 0 and h == 0),
                                stop=(kc == 15 and j == 3 and h == 1))
                    if kc == 15:
                        obf = stage.tile([128, 512], bf16, tag="obf", name="obf")
                        nc.vector.tensor_copy(obf[:], psO[0][:])
                        return obf
                    return None

                def emit_transposes(obf):
                    psT = mpsum.tile([128, 512], f32, tag="m",
                                     name="psT")[:].bitcast(bf16)
                    for j in range(4):
                        nc.tensor.matmul(psT[:, 128 * j:128 * (j + 1)],
                                         lhsT=obf[:, 128 * j:128 * (j + 1)],
                                         rhs=idb[:], is_transpose=True,
                                         start=(j == 0), stop=(j == 3),
                                         skip_group_check=True)
                    return psT

                def emit_ot_copy(p, qc, psT):
                    qs = slice(512 * qc, 512 * (qc + 1))
                    nc.vector.tensor_copy(ot[p][:, qs], psT[:, 0:512])

                def attn_v_mova(p, qc, kc, a):
                    if kc == 0:
                        psO[0] = opsum.tile([64, 1024], f32, tag="o2", name="psO2")
                    vb = v[kc // 2][:, 256 * (kc % 2):256 * (kc % 2 + 1)]
                    mmb(psO[0][0:64, 0:512], vb[:, 128 * p:128 * p + 64],
                        a[:, 0:512], start=(kc == 0), stop=(kc == 15))
                    mmb(psO[0][0:64, 512:1024],
                        vb[:, 128 * p + 64:128 * p + 128],
                        a[:, 512:1024], start=(kc == 0), stop=(kc == 15))
                    if kc == 15:
                        qs = slice(512 * qc, 512 * (qc + 1))
                        nc.vector.tensor_copy(ot[p][0:64, qs], psO[0][0:64, 0:512])
                        tmp = otmp[4 * p + qc]
                        nc.vector.tensor_copy(tmp[0:64, :], psO[0][0:64, 512:1024])
                        nc.sync.dma_start(out=ot[p][64:128, qs], in_=tmp[0:64, :])

                attn_v = attn_v_movv if ATTNV == "movv" else attn_v_mova

                wave_n = [0]

                def emit_wave(qc, w):
                    # tokens 512qc+128w : out rows <- ot[:, block] @ wo
                    tk = slice(512 * qc + 128 * w, 512 * qc + 128 * (w + 1))
                    psW = mpsum.tile([128, 512], f32, tag="m", name="psW")
                    for c in range(2):
                        mmb(psW[:], ot[c][:, tk], wo_sb[:, 512 * c:512 * (c + 1)],
                            start=(c == 0), stop=(c == 1))
                    st = stage.tile([128, 512], f32, tag="st", name="st")
                    nc.vector.tensor_copy(st[:], psW[:])
                    wave_n[0] += 1
                    nc.sync.dma_start(out=out_d[tk, :], in_=st[:])

                # ---- main software-pipelined loop ----
                flat = [(qc, p, kc) for qc in range(4) for p in range(2)
                        for kc in range(16)]
                T = len(flat)
                dve_set = {kc for kc in range(16)
                           if (kc + 1) * NDVE // 16 > kc * NDVE // 16}

                # prologue: only the chains scores(0) needs; rest are
                # emitted just-before-deadline inside the loop.
                proj_chain(0, 0, 0)   # qt0 qc0
                proj_chain(1, 0, 0)   # kt0 qc0
                chain_deadlines = (
                    [(1, 0, qcK, 4 * qcK) for qcK in range(1, 4)] +
                    [(0, 1, 0, 16)] +
                    [(1, 1, qcK, 16 + 4 * qcK) for qcK in range(4)] +
                    [(0, 0, 1, 32), (0, 1, 1, 48), (0, 0, 2, 64),
                     (0, 1, 2, 80), (0, 0, 3, 96), (0, 1, 3, 112)])
                sched = {}
                for which, mc, qcK, dl in chain_deadlines:
                    sched.setdefault(max(0, dl - 4), []).append(
                        (lambda w=which, m=mc, q=qcK: proj_chain(w, m, q)))

                attn_q = {}
                y_pend = {}
                obf_pend = {}
                ot_pend = {}
                wave_q = []
                for i in range(T + DELAY + 8):
                    # 1. attnV for the delayed tile (PE) + group-end obf (DVE)
                    j = i - DELAY
                    if DELAY <= i < T + DELAY:
                        qc, p, kc = flat[j]
                        obf = attn_v(p, qc, kc, attn_q.pop(j))
                        if obf is not None:
                            obf_pend[i + 1] = (p, qc, obf)
                            if p == 1:
                                wave_q.extend((i + 3 + 2 * w, qc, w)
                                              for w in range(4))
                    # 2. finish DVE sigmoid of previous tile (DVE ready now)
                    if i - 1 in y_pend:
                        attn_q[i - 1] = emit_sigmoid_dve3(y_pend.pop(i - 1))
                    # 3. new scores + sigmoid start
                    if i < T:
                        qc, p, kc = flat[i]
                        s = emit_scores(p, qc, kc)
                        if kc in dve_set:
                            y_pend[i] = emit_sigmoid_dve12(s)
                        else:
                            attn_q[i] = emit_sigmoid_act(s)
                    # 4. group-end pipeline: transposes, then ot copy
                    if i in obf_pend:
                        p2, q2, o2 = obf_pend.pop(i)
                        ot_pend[i + 1] = (p2, q2, emit_transposes(o2))
                    if i in ot_pend:
                        emit_ot_copy(*ot_pend.pop(i))
                    # 5. scheduled proj chains
                    for fn in sched.pop(i, []):
                        fn()
                    # 6. V projection pairs (first 16 tiles, every other)
                    if i < 16 and i % 2 == 0:
                        emit_v_pair(i // 2)
                    # 7. output waves
                    if wave_q and i >= wave_q[0][0]:
                        emit_wave(*wave_q.pop(0)[1:])
                while wave_q:
                    emit_wave(*wave_q.pop(0)[1:])

                if DEBUG:
                    for m in range(2):
                        nc.sync.dma_start(out=dbg["qt"][m], in_=qt[m][:])
                        nc.sync.dma_start(out=dbg["kt"][m], in_=kt[m][:])
                    for m in range(2):
                        otf = persist.tile([128, 2048], f32, tag=f"otf{m}",
                                           name=f"otf{m}")
                        nc.vector.tensor_copy(otf[:], ot[m][:])
                        nc.sync.dma_start(out=dbg["ot"][m], in_=otf[:])
                    for t in range(8):
                        vf = persist.tile([128, 512], f32, tag=f"vf{t}",
                                          name=f"vf{t}")
                        nc.vector.tensor_copy(vf[:], v[t][:])
                        nc.sync.dma_start(out=dbg["v"][2 * t], in_=vf[:, 0:256])
                        nc.sync.dma_start(out=dbg["v"][2 * t + 1],
                                          in_=vf[:, 256:512])

    nc.compile()
    return nc


def get_nc():
    if "nc" not in _CACHE:
        _CACHE["nc"] = _build_nc()
    return _CACHE["nc"]


def make_in_maps(x, W_q, W_k, W_v, W_o):
    x = np.ascontiguousarray(np.asarray(x, dtype=np.float32))
    ws = [np.asarray(w, dtype=np.float32) for w in (W_q, W_k, W_v, W_o)]
    W_q, W_k, W_v, W_o = ws

    def chunked(a, nchunks):
        # [128*nchunks, m] -> [128, nchunks*m] with chunk-major columns
        m = a.shape[1]
        return np.ascontiguousarray(
            a.reshape(nchunks, 128, m).transpose(1, 0, 2).reshape(128, nchunks * m))

    ident = np.ascontiguousarray(np.eye(128, dtype=np.float32))
    in_maps = []
    for c in range(8):
        b, g = divmod(c, 2)
        gf = slice(GF * g, GF * (g + 1))
        in_maps.append({
            "xt": chunked(np.ascontiguousarray(x[b].T), 4),
            "wq": chunked(np.ascontiguousarray(W_q[gf, :].T), 4),
            "wk": chunked(np.ascontiguousarray(W_k[gf, :].T), 4),
            "wv": chunked(np.ascontiguousarray(W_v[gf, :].T), 4),
            "wo": chunked(np.ascontiguousarray(W_o[:, gf].T), 2),
            "ident": ident,
        })
    return in_maps


def kernel(x, W_q, W_k, W_v, W_o):
    from concourse.bass_utils import run_bass_kernel_spmd

    nc = get_nc()
    in_maps = make_in_maps(x, W_q, W_k, W_v, W_o)
    res = run_bass_kernel_spmd(nc, in_maps, list(range(8)))
    parts = [res.results[c]["out"] for c in range(8)]
    out = np.stack([parts[2 * b] + parts[2 * b + 1] for b in range(B)])
    return np.ascontiguousarray(out.astype(np.float32))


# revision 26
# speedup vs baseline: 13.5612x; 1.0768x over previous
"""Sigmoid-attention MHA kernel for 8 Trainium2 NeuronCores (v2).

Problem: x[4,2048,512], W_q/W_k/W_v/W_o[512,512] (already scaled).
  Q = x@Wq.T, K = x@Wk.T, V = x@Wv.T split into 8 heads of depth 64
  attn = sigmoid(QK^T/sqrt(64) - log(2048));  out = (attn@V merged)@Wo.T

Sharding: core c handles batch b=c//2, head-group g=c%2 (4 heads each).
Each core computes a partial output projection over its 256 head-features;
host sums the two partials per batch.

v2 engine plan (per core):
  PE      scores (fp32r, 2 heads/tile), attn@V as attn-stationary x
          V-moving bf16 (64-row moving passes), O-block transposes,
          Q/K/V/Wo projections.
  ScalarE sigmoid on ~(16-NDVE)/16 of score tiles (PSUM->SBUF bf16).
  Pool    Schraudolph affine->int32 (e^-z bits) for the DVE tiles,
          psO->bf16 staging copies, Wo wave copies.
  DVE     +1.0 and fused reciprocal (custom op) to finish sigmoid on
          NDVE/16 tiles, psT->ot copies, projection copies.
  attn@V is emitted DELAY tiles behind scores so the long DVE sigmoid
  latency never stalls the PE.
"""

import os
import numpy as np

DEBUG = bool(int(os.environ.get("KERNEL_DEBUG", "0")))
LOOP = int(os.environ.get("KERNEL_LOOP", "0"))  # >0: wrap body in For_i (timing)
NDVE = int(os.environ.get("KERNEL_NDVE", "0"))  # kc%16 < NDVE -> DVE sigmoid
DELAY = int(os.environ.get("KERNEL_DELAY", "5"))  # attnV emission delay (tiles)
ATTNV = os.environ.get("KERNEL_ATTNV", "movv")  # movv | mova
I2ENG = os.environ.get("KERNEL_I2ENG", "pool")  # dve | pool
ABUFS = int(os.environ.get("KERNEL_ABUFS", "6"))
GPDMA = bool(int(os.environ.get("KERNEL_GPDMA", "0")))
SRESET = bool(int(os.environ.get("KERNEL_SRESET", "0")))

B, S, D = 4, 2048, 512
NH, DEPTH = 8, 64
G = 2          # head groups (one per core pair)
GF = 256       # features per group
NEG_LOG_S = float(np.float32(-np.log(np.float32(S))))
INV_SQRT_DK = 0.125

# Schraudolph constants: bits(e^-z) ~= round(s*SC0 + SC1), z = s/8 - log(S)
A_LN2 = float(2**23 / np.log(2))
CMAGIC = 486411.0
SC0 = float(np.float32(-A_LN2 / 8.0))
SC1 = float(np.float32(A_LN2 * np.log(float(S)) + 127 * 2**23 - CMAGIC))

_CACHE = {}


def _build_nc():
    import concourse.bacc as bacc
    import concourse.tile as tile
    from concourse import mybir
    from concourse.alu_op_type import AluOpType
    from concourse.dve_ops import RECIP_APPROX_FAST_CONSTS, RECIPROCAL_APPROX_FAST

    f32 = mybir.dt.float32
    f32r = mybir.dt.float32r
    bf16 = mybir.dt.bfloat16
    i32 = mybir.dt.int32
    RC = RECIP_APPROX_FAST_CONSTS
    nc = bacc.Bacc("TRN2", target_bir_lowering=False, debug=False, num_devices=8)

    xt_d = nc.dram_tensor("xt", [128, 8192], f32r, kind="ExternalInput").ap()
    wq_d = nc.dram_tensor("wq", [128, 1024], f32r, kind="ExternalInput").ap()
    wk_d = nc.dram_tensor("wk", [128, 1024], f32r, kind="ExternalInput").ap()
    wv_d = nc.dram_tensor("wv", [128, 1024], f32r, kind="ExternalInput").ap()
    wo_d = nc.dram_tensor("wo", [128, 1024], f32, kind="ExternalInput").ap()
    id_d = nc.dram_tensor("ident", [128, 128], f32, kind="ExternalInput").ap()
    out_d = nc.dram_tensor("out", [S, D], f32, kind="ExternalOutput").ap()
    dbg = {}
    if DEBUG:
        for nm in ("qt", "kt"):
            dbg[nm] = [nc.dram_tensor(f"dbg_{nm}{m}", [128, 2048], f32r,
                                      kind="ExternalOutput").ap() for m in range(2)]
        dbg["ot"] = [nc.dram_tensor(f"dbg_ot{m}", [128, 2048], f32,
                                    kind="ExternalOutput").ap() for m in range(2)]
        dbg["v"] = [nc.dram_tensor(f"dbg_v{t}", [128, 256], f32,
                                   kind="ExternalOutput").ap() for t in range(16)]

    with tile.TileContext(nc) as tc:
        with (
            tc.tile_pool(name="persist", bufs=1) as persist,
            tc.tile_pool(name="attn", bufs=ABUFS) as apool,
            tc.tile_pool(name="epool", bufs=3) as epool,
            tc.tile_pool(name="stage", bufs=3) as stage,
            tc.tile_pool(name="spsum", bufs=3 if ATTNV == "movv" else 2,
                         space="PSUM") as spsum,
            tc.tile_pool(name="opsum", bufs=1, space="PSUM") as opsum,
            tc.tile_pool(name="mpsum", bufs=1, space="PSUM") as mpsum,
        ):
            import contextlib
            if LOOP > 0:
                loop_cm = tc.For_i(0, LOOP, 1, staggered_reset=SRESET)
            else:
                loop_cm = contextlib.nullcontext()
            Sig = mybir.ActivationFunctionType.Sigmoid

            def mm(out, lhsT, rhs, start, stop):
                # f32r: single-pass fp32 matmul (4x faster than fp32 on PE)
                nc.tensor.matmul(out, lhsT=lhsT.bitcast(f32r),
                                 rhs=rhs.bitcast(f32r), start=start, stop=stop)

            def mmb(out, lhsT, rhs, start, stop):
                nc.tensor.matmul(out, lhsT=lhsT, rhs=rhs, start=start,
                                 stop=stop, skip_group_check=True)

            with loop_cm:
                bias_t = persist.tile([128, 1], f32, tag="bias", name="bias_t")
                nc.vector.memset(bias_t[:], NEG_LOG_S)
                warm_t = persist.tile([128, 1], f32, tag="warm", name="warm_t")
                nc.scalar.activation(warm_t[:], bias_t[:], Sig, bias=bias_t[:])

                wq_sb = persist.tile([128, 1024], f32r, tag="wq", name="wq_sb")
                wk_sb = persist.tile([128, 1024], f32r, tag="wk", name="wk_sb")
                wv_sb = persist.tile([128, 1024], f32r, tag="wv", name="wv_sb")
                wo_f = persist.tile([128, 1024], f32, tag="wof", name="wo_f")
                wo_sb = persist.tile([128, 1024], bf16, tag="wo", name="wo_sb")
                id_f = persist.tile([128, 128], f32, tag="idf", name="id_f")
                idb = persist.tile([128, 128], bf16, tag="idb", name="idb")
                xt = [persist.tile([128, 2048], f32r, tag=f"xt{c}", name=f"xt{c}")
                      for c in range(4)]
                q3 = nc.gpsimd if GPDMA else nc.sync
                nc.sync.dma_start(out=xt[0][:], in_=xt_d[:, 0:2048])
                nc.scalar.dma_start(out=xt[1][:], in_=xt_d[:, 2048:4096])
                q3.dma_start(out=xt[2][:], in_=xt_d[:, 4096:6144])
                nc.sync.dma_start(out=xt[3][:], in_=xt_d[:, 6144:8192])
                nc.scalar.dma_start(out=wq_sb[:], in_=wq_d[:])
                q3.dma_start(out=wk_sb[:], in_=wk_d[:])
                nc.sync.dma_start(out=wv_sb[:], in_=wv_d[:])
                nc.scalar.dma_start(out=wo_f[:], in_=wo_d[:])
                q3.dma_start(out=id_f[:], in_=id_d[:])
                nc.vector.tensor_copy(wo_sb[:], wo_f[:])
                nc.vector.tensor_copy(idb[:], id_f[:])

                qt = [persist.tile([128, 2048], f32r, tag=f"qt{m}", name=f"qt{m}")
                      for m in range(2)]
                kt = [persist.tile([128, 2048], f32r, tag=f"kt{m}", name=f"kt{m}")
                      for m in range(2)]
                v = [persist.tile([128, 512], bf16, tag=f"v{t}", name=f"v{t}")
                     for t in range(8)]
                ot = [persist.tile([128, 2048], bf16, tag=f"ot{m}", name=f"ot{m}")
                      for m in range(2)]
                otmp = [persist.tile([64, 512], bf16, tag=f"otmp{m}",
                                     name=f"otmp{m}") for m in range(8)]

                # ---- Q/K projection chain (emitted just-in-time) ----
                pi = [0]

                def proj_chain(which, mc, qc):
                    w_sb = (wq_sb, wk_sb)[which]
                    dst = (qt, kt)[which][mc]
                    ps = mpsum.tile([128, 512], f32, tag="m", name="psP")
                    for kc in range(4):
                        mm(ps[:, 0:512],
                           w_sb[:, 256 * kc + 128 * mc:256 * kc + 128 * mc + 128],
                           xt[kc][:, 512 * qc:512 * (qc + 1)],
                           start=(kc == 0), stop=(kc == 3))
                    nc.vector.tensor_copy(
                        dst[:, 512 * qc:512 * (qc + 1)], ps[:, 0:512])
                    pi[0] += 1

                # ---- attention ----
                def emit_v_pair(t2):
                    # V proj for token chunks 2*t2, 2*t2+1 into one psum slot
                    pv = spsum.tile([128, 1024], f32, tag="s", name="ps")
                    for half in range(2):
                        tck = 2 * t2 + half
                        cs = slice(256 * half, 256 * (half + 1))
                        for vkc in range(4):
                            mm(pv[:, cs],
                               xt[vkc][:, 128 * tck:128 * (tck + 1)],
                               wv_sb[:, 256 * vkc:256 * (vkc + 1)],
                               start=(vkc == 0 and half == 0),
                               stop=(vkc == 3 and half == 1))
                    nc.vector.tensor_copy(v[t2][:], pv[:, 0:512])

                def emit_scores(p, qc, kc):
                    ks = slice(128 * kc, 128 * (kc + 1))
                    qs = slice(512 * qc, 512 * (qc + 1))
                    s = spsum.tile([128, 1024], f32, tag="s", name="ps")
                    mm(s[:, 0:512], kt[p][0:64, ks], qt[p][0:64, qs],
                       start=True, stop=True)
                    mm(s[:, 512:1024], kt[p][64:128, ks],
                       qt[p][64:128, qs], start=True, stop=True)
                    return s

                def emit_sigmoid_act(s):
                    a = apool.tile([128, 1024], bf16, tag="a", name="attn")
                    nc.scalar.activation(a[:], s[:], Sig,
                                         bias=bias_t[:], scale=INV_SQRT_DK)
                    return a

                def emit_sigmoid_dve12(s):
                    # e^{-z} bits via Schraudolph affine -> int32, then +1.0
                    ei = epool.tile([128, 1024], i32, tag="ei", name="ei")
                    nc.vector.tensor_scalar(ei[:], s[:], SC0, SC1,
                                            AluOpType.mult, AluOpType.add)
                    y = epool.tile([128, 1024], f32, tag="y", name="y")
                    i2 = nc.vector if I2ENG == "dve" else nc.gpsimd
                    i2.tensor_scalar_add(y[:], ei[:].bitcast(f32), 1.0)
                    return y

                def emit_sigmoid_dve3(y):
                    # sigma = 1/(1+e^{-z})
                    a = apool.tile([128, 1024], bf16, tag="a", name="attn")
                    nc.vector._custom_dve(RECIPROCAL_APPROX_FAST, out=a[:],
                                          in0=y[:], s0=RC["s0"],
                                          s1=RC["s1"], imm2=RC["imm2"])
                    return a

                psO = [None]

                def attn_v_movv(p, qc, kc, a):
                    # returns obf staging tile at group end (kc==15)
                    if kc == 0:
                        psO[0] = opsum.tile([128, 512], f32, tag="o", name="psO")
                    for j in range(4):
                        for h in range(2):
                            mmb(psO[0][:, 128 * j + 64 * h:128 * j + 64 * h + 64],
                                a[:, 512 * h + 128 * j:512 * h + 128 * j + 128],
                                v[kc // 2][:, 256 * (kc % 2) + 128 * p + 64 * h:
                                            256 * (kc % 2) + 128 * p + 64 * h + 64],
                                start=(kc == 0 and j == 0 and h == 0),
                                stop=(kc == 15 and j == 3 and h == 1))
                    if kc == 15:
                        obf = stage.tile([128, 512], bf16, tag="obf", name="obf")
                        nc.vector.tensor_copy(obf[:], psO[0][:])
                        return obf
                    return None

                def emit_transposes(obf):
                    psT = mpsum.tile([128, 512], f32, tag="m",
                                     name="psT")[:].bitcast(bf16)
                    for j in range(4):
                        nc.tensor.matmul(psT[:, 128 * j:128 * (j + 1)],
                                         lhsT=obf[:, 128 * j:128 * (j + 1)],
                                         rhs=idb[:], is_transpose=True,
                                         start=(j == 0), stop=(j == 3),
                                         skip_group_check=True)
                    return psT

                def emit_ot_copy(p, qc, psT):
                    qs = slice(512 * qc, 512 * (qc + 1))
                    nc.vector.tensor_copy(ot[p][:, qs], psT[:, 0:512])

                def attn_v_mova(p, qc, kc, a):
                    if kc == 0:
                        psO[0] = opsum.tile([64, 1024], f32, tag="o2", name="psO2")
                    vb = v[kc // 2][:, 256 * (kc % 2):256 * (kc % 2 + 1)]
                    mmb(psO[0][0:64, 0:512], vb[:, 128 * p:128 * p + 64],
                        a[:, 0:512], start=(kc == 0), stop=(kc == 15))
                    mmb(psO[0][0:64, 512:1024],
                        vb[:, 128 * p + 64:128 * p + 128],
                        a[:, 512:1024], start=(kc == 0), stop=(kc == 15))
                    if kc == 15:
                        qs = slice(512 * qc, 512 * (qc + 1))
                        nc.vector.tensor_copy(ot[p][0:64, qs], psO[0][0:64, 0:512])
                        tmp = otmp[4 * p + qc]
                        nc.vector.tensor_copy(tmp[0:64, :], psO[0][0:64, 512:1024])
                        nc.sync.dma_start(out=ot[p][64:128, qs], in_=tmp[0:64, :])

                attn_v = attn_v_movv if ATTNV == "movv" else attn_v_mova

                wave_n = [0]

                def emit_wave(qc, w):
                    # tokens 512qc+128w : out rows <- ot[:, block] @ wo
                    tk = slice(512 * qc + 128 * w, 512 * qc + 128 * (w + 1))
                    psW = mpsum.tile([128, 512], f32, tag="m", name="psW")
                    for c in range(2):
                        mmb(psW[:], ot[c][:, tk], wo_sb[:, 512 * c:512 * (c + 1)],
                            start=(c == 0), stop=(c == 1))
                    st = stage.tile([128, 512], f32, tag="st", name="st")
                    nc.vector.tensor_copy(st[:], psW[:])
                    wave_n[0] += 1
                    nc.sync.dma_start(out=out_d[tk, :], in_=st[:])

                # ---- main software-pipelined loop ----
                flat = [(qc, p, kc) for qc in range(4) for p in range(2)
                        for kc in range(16)]
                T = len(flat)
                dve_set = {kc for kc in range(16)
                           if (kc + 1) * NDVE // 16 > kc * NDVE // 16}

                # prologue: only the chains scores(0) needs; rest are
                # emitted just-before-deadline inside the loop.
                proj_chain(0, 0, 0)   # qt0 qc0
                proj_chain(1, 0, 0)   # kt0 qc0
                chain_deadlines = (
                    [(1, 0, qcK, 4 * qcK) for qcK in range(1, 4)] +
                    [(0, 1, 0, 16)] +
                    [(1, 1, qcK, 16 + 4 * qcK) for qcK in range(4)] +
                    [(0, 0, 1, 32), (0, 1, 1, 48), (0, 0, 2, 64),
                     (0, 1, 2, 80), (0, 0, 3, 96), (0, 1, 3, 112)])
                sched = {}
                for which, mc, qcK, dl in chain_deadlines:
                    sched.setdefault(max(0, dl - 4), []).append(
                        (lambda w=which, m=mc, q=qcK: proj_chain(w, m, q)))

                attn_q = {}
                y_pend = {}
                obf_pend = {}
                ot_pend = {}
                wave_q = []
                for i in range(T + DELAY + 8):
                    # 1. attnV for the delayed tile (PE) + group-end obf (DVE)
                    j = i - DELAY
                    if DELAY <= i < T + DELAY:
                        qc, p, kc = flat[j]
                        obf = attn_v(p, qc, kc, attn_q.pop(j))
                        if obf is not None:
                            obf_pend[i + 1] = (p, qc, obf)
                            if p == 1:
                                wave_q.extend((i + 3 + 2 * w, qc, w)
                                              for w in range(4))
                    # 2. finish DVE sigmoid of previous tile (DVE ready now)
                    if i - 1 in y_pend:
                        attn_q[i - 1] = emit_sigmoid_dve3(y_pend.pop(i - 1))
                    # 3. new scores + sigmoid start
                    if i < T:
                        qc, p, kc = flat[i]
                        s = emit_scores(p, qc, kc)
                        if kc in dve_set:
                            y_pend[i] = emit_sigmoid_dve12(s)
                        else:
                            attn_q[i] = emit_sigmoid_act(s)
                    # 4. group-end pipeline: transposes, then ot copy
                    if i in obf_pend:
                        p2, q2, o2 = obf_pend.pop(i)
                        ot_pend[i + 1] = (p2, q2, emit_transposes(o2))
                    if i in ot_pend:
                        emit_ot_copy(*ot_pend.pop(i))
                    # 5. scheduled proj chains
                    for fn in sched.pop(i, []):
                        fn()
                    # 6. V projection pairs (first 16 tiles, every other)
                    if i < 16 and i % 2 == 0:
                        emit_v_pair(i // 2)
                    # 7. output waves
                    if wave_q and i >= wave_q[0][0]:
                        emit_wave(*wave_q.pop(0)[1:])
                while wave_q:
                    emit_wave(*wave_q.pop(0)[1:])

                if DEBUG:
                    for m in range(2):
                        nc.sync.dma_start(out=dbg["qt"][m], in_=qt[m][:])
                        nc.sync.dma_start(out=dbg["kt"][m], in_=kt[m][:])
                    for m in range(2):
                        otf = persist.tile([128, 2048], f32, tag=f"otf{m}",
                                           name=f"otf{m}")
                        nc.vector.tensor_copy(otf[:], ot[m][:])
                        nc.sync.dma_start(out=dbg["ot"][m], in_=otf[:])
                    for t in range(8):
                        vf = persist.tile([128, 512], f32, tag=f"vf{t}",
                                          name=f"vf{t}")
                        nc.vector.tensor_copy(vf[:], v[t][:])
                        nc.sync.dma_start(out=dbg["v"][2 * t], in_=vf[:, 0:256])
                        nc.sync.dma_start(out=dbg["v"][2 * t + 1],
                                          in_=vf[:, 256:512])

    nc.compile()
    return nc


def get_nc():
    if "nc" not in _CACHE:
        _CACHE["nc"] = _build_nc()
    return _CACHE["nc"]


def make_in_maps(x, W_q, W_k, W_v, W_o):
    x = np.ascontiguousarray(np.asarray(x, dtype=np.float32))
    ws = [np.asarray(w, dtype=np.float32) for w in (W_q, W_k, W_v, W_o)]
    W_q, W_k, W_v, W_o = ws

    def chunked(a, nchunks):
        # [128*nchunks, m] -> [128, nchunks*m] with chunk-major columns
        m = a.shape[1]
        return np.ascontiguousarray(
            a.reshape(nchunks, 128, m).transpose(1, 0, 2).reshape(128, nchunks * m))

    ident = np.ascontiguousarray(np.eye(128, dtype=np.float32))
    in_maps = []
    for c in range(8):
        b, g = divmod(c, 2)
        gf = slice(GF * g, GF * (g + 1))
        in_maps.append({
            "xt": chunked(np.ascontiguousarray(x[b].T), 4),
            "wq": chunked(np.ascontiguousarray(W_q[gf, :].T), 4),
            "wk": chunked(np.ascontiguousarray(W_k[gf, :].T), 4),
            "wv": chunked(np.ascontiguousarray(W_v[gf, :].T), 4),
            "wo": chunked(np.ascontiguousarray(W_o[:, gf].T), 2),
            "ident": ident,
        })
    return in_maps


def kernel(x, W_q, W_k, W_v, W_o):
    from concourse.bass_utils import run_bass_kernel_spmd

    nc = get_nc()
    in_maps = make_in_maps(x, W_q, W_k, W_v, W_o)
    res = run_bass_kernel_spmd(nc, in_maps, list(range(8)))
    parts = [res.results[c]["out"] for c in range(8)]
    out = np.stack([parts[2 * b] + parts[2 * b + 1] for b in range(B)])
    return np.ascontiguousarray(out.astype(np.float32))
